# revision 1
# baseline (speedup 1.0000x reference)
"""Trainium2 Bass kernel for nn_MultiHeadAttention (B=4, T=2048, D=2048, H=16).

Sharding: tensor-parallel over heads. Each of 8 NeuronCores owns 2 heads
(256 of the 2048 Q/K/V dims). Per core:
  phase 1: qT/kT projections in transposed layout [head_dim, tokens] and v in
           normal layout [tokens, head_dim] (so attention needs no on-chip
           transposes), streaming xT from HBM.
  phase 2: per (batch, head): scoresT[ktok, qtok] = kT_chunk.T @ qT, exp
           (no max-subtraction -- logits are O(1) by construction), causal
           block-skip + diagonal-block masks, AV accumulation into
           unnormalized ctxT, softmax denominator via DVE tree-add + a
           ones-vector matmul.
  phase 3: out_partial = sum_h (1/den_h)[token] * (ctxT_h.T @ WoT_h), the
           per-token normalization applied via per-partition activation scale.
Host: Wo partials summed across cores; k/v slices concatenated.
Matmuls run as float32r (full PE rate for 4-byte floats at free-dim >= 256).
"""

import os
import sys

import numpy as np

for _p in ("/opt/trn_rl_repo",):
    if _p not in sys.path and os.path.isdir(_p):
        sys.path.insert(0, _p)

B, T, D, H = 4, 2048, 2048, 16
HD = 128
N_CORES = 8
HPC = H // N_CORES          # heads per core
DPC = HPC * HD              # q/k/v dims per core
NTOK = B * T

P = 128
QT = 512                    # q-tile width
KC = 128                    # k-chunk
PT = 512                    # phase-1 token tile
DSUB = 4                    # d-chunks per streamed xT tile
DIAG = QT // KC

_CACHE = {}


def _build_module():
    import concourse.bass as bass  # noqa: F401
    import concourse.mybir as mybir
    from concourse import bacc
    import concourse.tile as tile

    F32 = mybir.dt.float32
    F32R = mybir.dt.float32r
    AF = mybir.ActivationFunctionType
    ALU = mybir.AluOpType

    def cast(ap):
        return ap

    DK = D // P
    TBLK = NTOK // P
    NPT = NTOK // PT
    NQT = T // QT
    SCALE = 1.0 / float(np.sqrt(HD))

    nc = bacc.Bacc("TRN2", target_bir_lowering=False, debug=False)

    xT = nc.dram_tensor("xT", [D, NTOK], F32, kind="ExternalInput").ap()
    wqT = nc.dram_tensor("wqT", [D, DPC], F32, kind="ExternalInput").ap()
    wkT = nc.dram_tensor("wkT", [D, DPC], F32, kind="ExternalInput").ap()
    wvT = nc.dram_tensor("wvT", [D, DPC], F32, kind="ExternalInput").ap()
    woT = nc.dram_tensor("woT", [DPC, D], F32, kind="ExternalInput").ap()
    masks = nc.dram_tensor("masks", [DIAG, KC, QT], F32, kind="ExternalInput").ap()

    kT_out = nc.dram_tensor("kT_out", [DPC, NTOK], F32, kind="ExternalOutput").ap()
    v_out = nc.dram_tensor("v_out", [NTOK, DPC], F32, kind="ExternalOutput").ap()
    out_p = nc.dram_tensor("out_p", [NTOK, D], F32, kind="ExternalOutput").ap()

    xT_v = xT.rearrange("(dk p) t -> p dk t", p=P)
    wqT_v = wqT.rearrange("(dk p) n -> p dk n", p=P)
    wkT_v = wkT.rearrange("(dk p) n -> p dk n", p=P)
    wvT_v = wvT.rearrange("(dk p) n -> p dk n", p=P)
    woT_v = woT.rearrange("(hc p) n -> p hc n", p=P)
    masks_v = masks.rearrange("j p q -> p j q")
    v_out_v = v_out.rearrange("(c p) n -> p c n", p=P)

    with tile.TileContext(nc) as tc:
        with tc.tile_pool(name="dram", bufs=1, space="DRAM") as dpool:
            q_scr = dpool.tile([HPC, P, NTOK], F32)

            # ---------------- Phase 1: projections ----------------
            with (
                tc.tile_pool(name="wq", bufs=1) as wq_pool,
                tc.tile_pool(name="xt", bufs=2 * (DK // DSUB)) as xt_pool,
                tc.tile_pool(name="st1", bufs=3) as st_pool,
                tc.tile_pool(name="pp_qk", bufs=2, space="PSUM") as pp_qk,
                tc.tile_pool(name="pp_v", bufs=2, space="PSUM") as pp_v,
            ):
                def load_xt(tb):
                    ts = slice(tb * PT, (tb + 1) * PT)
                    xts = []
                    for dg in range(DK // DSUB):
                        xt_t = xt_pool.tile([P, DSUB, PT], F32R, tag="xt")
                        nc.sync.dma_start(
                            xt_t[:], xT_v[:, dg * DSUB:(dg + 1) * DSUB, ts].bitcast(F32R))
                        xts.append(xt_t)
                    return xts

                # first xT tile + Wq first so the PE starts ASAP;
                # Wk/Wv land while the q-projection of tb=0 runs
                wq_sb = wq_pool.tile([P, DK, DPC], F32R, tag="wq")
                wk_sb = wq_pool.tile([P, DK, DPC], F32R, tag="wk")
                wv_sb = wq_pool.tile([P, DK, DPC], F32R, tag="wv")
                nc.sync.dma_start(wq_sb[:], wqT_v.bitcast(F32R))
                xts0 = load_xt(0)
                nc.sync.dma_start(wk_sb[:], wkT_v.bitcast(F32R))
                nc.sync.dma_start(wv_sb[:], wvT_v.bitcast(F32R))

                for tb in range(NPT):
                    xts = xts0 if tb == 0 else load_xt(tb)
                    ts = slice(tb * PT, (tb + 1) * PT)

                    def xchunk(dc):
                        return xts[dc // DSUB][:, dc % DSUB, :]

                    for w_sb, is_q in ((wq_sb, True), (wk_sb, False)):
                        for hc in range(HPC):
                            ps = pp_qk.tile([P, PT], F32, tag="pqk")
                            for dc in range(DK):
                                nc.tensor.matmul(
                                    ps[:],
                                    cast(w_sb[:, dc, hc * P:(hc + 1) * P]),
                                    cast(xchunk(dc)),
                                    start=(dc == 0), stop=(dc == DK - 1))
                            st = st_pool.tile([P, PT], F32, tag="stqk")
                            nc.vector.tensor_copy(st[:], ps[:])
                            if is_q:
                                nc.sync.dma_start(q_scr[hc, :, ts], st[:])
                            else:
                                nc.sync.dma_start(
                                    kT_out[hc * P:(hc + 1) * P, ts], st[:])

                    for sub in range(PT // P):
                        t0 = tb * PT + sub * P
                        ps = pp_v.tile([P, DPC], F32, tag="pv")
                        for dc in range(DK):
                            nc.tensor.matmul(
                                ps[:],
                                cast(xchunk(dc)[:, sub * P:(sub + 1) * P]),
                                cast(wv_sb[:, dc, :]),
                                start=(dc == 0), stop=(dc == DK - 1))
                        st = st_pool.tile([P, DPC], F32, tag="stv")
                        nc.vector.tensor_copy(st[:], ps[:])
                        nc.sync.dma_start(v_out[t0:t0 + P, :], st[:])

            # ---------------- Phase 2+3: attention & output ----------------
            with tc.tile_pool(name="res", bufs=1) as res_pool:
                ctx_res = res_pool.tile([P, HPC, NTOK], F32R, tag="ctx")

                with (
                    tc.tile_pool(name="pair", bufs=2) as pair_pool,
                    tc.tile_pool(name="exp", bufs=6) as exp_pool,
                    tc.tile_pool(name="den", bufs=2) as den_pool,
                    tc.tile_pool(name="cst", bufs=1) as cst_pool,
                    tc.tile_pool(name="wo", bufs=1) as wo_pool,
                    tc.tile_pool(name="st3", bufs=3) as st3_pool,
                    tc.tile_pool(name="pp_s", bufs=2, space="PSUM") as pp_s,
                    tc.tile_pool(name="pp_ctx", bufs=2, space="PSUM") as pp_ctx,
                    tc.tile_pool(name="pp_den", bufs=2, space="PSUM") as pp_den,
                    tc.tile_pool(name="pp_o", bufs=2, space="PSUM") as pp_o,
                ):
                    wo_sb = wo_pool.tile([P, HPC, D], F32R, tag="wo")
                    nc.sync.dma_start(wo_sb[:], woT_v.bitcast(F32R))
                    mask_sb = cst_pool.tile([P, DIAG, QT], F32R, tag="mask")
                    ones_f = cst_pool.tile([P, P], F32, tag="onesf")
                    ones_sb = cst_pool.tile([P, P], F32R, tag="ones")
                    nc.sync.dma_start(mask_sb[:], masks_v.bitcast(F32R))
                    nc.vector.memset(ones_f[:], 1.0)
                    nc.vector.tensor_copy(ones_sb[:], ones_f[:])
                    def do_pair(b, h):
                        qt_pair = pair_pool.tile([P, T], F32R, tag="qpair")
                        kt_pair = pair_pool.tile([P, T], F32R, tag="kpair")
                        v_pair = pair_pool.tile([P, T // P, HD], F32R, tag="vpair")
                        bs = slice(b * T, (b + 1) * T)
                        nc.sync.dma_start(qt_pair[:], q_scr[h, :, bs].bitcast(F32R))
                        nc.sync.dma_start(
                            kt_pair[:], kT_out[h * P:(h + 1) * P, bs].bitcast(F32R))
                        nc.sync.dma_start(
                            v_pair[:],
                            v_out_v[:, b * (T // P):(b + 1) * (T // P),
                                    h * HD:(h + 1) * HD].bitcast(F32R))

                        for qt in range(NQT):
                            qs = slice(qt * QT, (qt + 1) * QT)
                            nkc = (qt + 1) * DIAG
                            ctx_ps = pp_ctx.tile([P, QT], F32, tag="pctx")
                            den_ps = pp_den.tile([P, QT], F32, tag="pden")
                            for kc in range(nkc):
                                s_ps = pp_s.tile([P, QT], F32, tag="ps")
                                nc.tensor.matmul(
                                    s_ps[:],
                                    cast(kt_pair[:, kc * KC:(kc + 1) * KC]),
                                    cast(qt_pair[:, qs]),
                                    start=True, stop=True)
                                e_t = exp_pool.tile([P, QT], F32R, tag="et")
                                nc.scalar.activation(
                                    e_t[:], s_ps[:], AF.Exp, scale=SCALE)
                                j = kc - qt * DIAG
                                if j >= 0:
                                    nc.vector.tensor_mul(
                                        e_t[:], e_t[:], mask_sb[:, j, :])
                                nc.tensor.matmul(
                                    ctx_ps[:],
                                    cast(v_pair[:, kc, :]),
                                    cast(e_t[:]),
                                    start=(kc == 0), stop=(kc == nkc - 1))
                                # denominator: ones[128,128] stationary sums
                                # e_t over ktok, replicated to all partitions
                                nc.tensor.matmul(
                                    den_ps[:], cast(ones_sb[:]), cast(e_t[:]),
                                    start=(kc == 0), stop=(kc == nkc - 1))
                            recip_bc = den_pool.tile([P, QT], F32, tag="rbc")
                            nc.vector.reciprocal(recip_bc[:], den_ps[:])
                            nc.vector.tensor_mul(
                                ctx_res[:, h, b * T + qt * QT:
                                        b * T + (qt + 1) * QT],
                                ctx_ps[:], recip_bc[:])

                    # ---- Phase 3 (per batch): output projection ----
                    NOD = D // QT

                    def do_out_block(tb):
                        ts2 = slice(tb * P, (tb + 1) * P)
                        ost = st3_pool.tile([P, D], F32, tag="ost")
                        for od in range(NOD):
                            ods = slice(od * QT, (od + 1) * QT)
                            ps0 = pp_o.tile([P, QT], F32, tag="po0")
                            nc.tensor.matmul(
                                ps0[:], cast(ctx_res[:, 0, ts2]),
                                cast(wo_sb[:, 0, ods]), start=True, stop=False)
                            nc.tensor.matmul(
                                ps0[:], cast(ctx_res[:, 1, ts2]),
                                cast(wo_sb[:, 1, ods]), start=False, stop=True)
                            if od % 2 == 0:
                                nc.vector.tensor_copy(ost[:, ods], ps0[:])
                            else:
                                nc.scalar.copy(ost[:, ods], ps0[:])
                        nc.sync.dma_start(out_p[ts2, :], ost[:])

                    for b in range(B):
                        for h in range(HPC):
                            do_pair(b, h)
                    for tb in range(TBLK):
                        do_out_block(tb)

    nc.compile()
    return nc


def _get_module():
    if "nc" not in _CACHE:
        _CACHE["nc"] = _build_module()
    return _CACHE["nc"]


def _make_masks():
    m = np.zeros((DIAG, KC, QT), dtype=np.float32)
    for j in range(DIAG):
        for kk in range(KC):
            m[j, kk, j * KC + kk:] = 1.0
    return m


def _run(x, Wq, Wk, Wv, Wo, bo, trace=False):
    from concourse import bass_utils

    nc = _get_module()
    x = np.asarray(x, dtype=np.float32)
    xT = np.ascontiguousarray(x.reshape(NTOK, D).T)
    masks = _make_masks()
    Wq = np.asarray(Wq, np.float32)
    Wk = np.asarray(Wk, np.float32)
    Wv = np.asarray(Wv, np.float32)
    Wo = np.asarray(Wo, np.float32)
    in_maps = []
    for c in range(N_CORES):
        sl = slice(c * DPC, (c + 1) * DPC)
        in_maps.append({
            "xT": xT,
            "wqT": np.ascontiguousarray(Wq[sl, :].T),
            "wkT": np.ascontiguousarray(Wk[sl, :].T),
            "wvT": np.ascontiguousarray(Wv[sl, :].T),
            "woT": np.ascontiguousarray(Wo[:, sl].T),
            "masks": masks,
        })
    res = bass_utils.run_bass_kernel_spmd(
        nc, in_maps, core_ids=list(range(N_CORES)), trace=trace)

    out = np.zeros((NTOK, D), np.float32)
    k = np.empty((NTOK, D), np.float32)
    v = np.empty((NTOK, D), np.float32)
    for c, r in enumerate(res.results):
        sl = slice(c * DPC, (c + 1) * DPC)
        out += r["out_p"]
        k[:, sl] = r["kT_out"].T
        v[:, sl] = r["v_out"]
    out += np.asarray(bo, np.float32)[None, :]
    outs = (out.reshape(B, T, D), k.reshape(B, T, D), v.reshape(B, T, D))
    return outs, res


def kernel(x, Wq, Wk, Wv, Wo, bo):
    outs, _ = _run(x, Wq, Wk, Wv, Wo, bo, trace=False)
    return outs



# revision 4
# speedup vs baseline: 1.3080x; 1.3080x over previous
"""Trainium2 Bass kernel for nn_MultiHeadAttention (B=4, T=2048, D=2048, H=16).

Sharding: tensor-parallel over heads; each of 8 NeuronCores owns 2 heads
(256 of 2048 q/k/v dims).  All matmul operands are bf16 (f32 PSUM
accumulation); inputs are cast host-side, outputs are written bf16 and
upcast/summed host-side.

Per core, three phases, kept fully SBUF-resident (no DRAM scratch):
  ph1(b): qT/kT projections in [head_dim, tok] layout and v in [tok, dim],
          streaming partition-contiguous x tiles from HBM.
  ph2(b): per (q-tile, head): scoresT[ktok, qtok] = kT_chunk.T @ qT, exp on
          ScalarE (no max-subtraction; logits are O(1) by construction),
          causal block-skip + diagonal masks on DVE, AV accumulation.
          Softmax denominator: DVE accumulates per-partition partial sums
          E[ktok%128, qtok] in fp16, then one ones-matmul per q-tile
          reduces over partitions (instead of a matmul per k-chunk).
  ph3(b): out_partial[tok, :] = sum_h (ctx_h/den_h).T @ WoT_h per
          128-token block, staged PSUM->SBUF (split DVE/ScalarE), written
          bf16.
The schedule interleaves three streams to keep the PE queue dense:
ph1(b+1) units alternate with ph2(b) groups, and ph3 blocks are drip-fed
into ph2's chunk loops to cover the exp (ScalarE) latency.
Host: Wo partials summed across cores in f32; k/v slices concatenated.
"""

import os
import sys

import numpy as np

for _p in ("/opt/trn_rl_repo",):
    if _p not in sys.path and os.path.isdir(_p):
        sys.path.insert(0, _p)

B, T, D, H = 4, 2048, 2048, 16
HD = 128
N_CORES = 8
HPC = H // N_CORES          # heads per core
DPC = HPC * HD              # q/k/v dims per core
NTOK = B * T

P = 128
QT = 512                    # q-tile width
KC = 128                    # k-chunk
PT = 512                    # phase-1 token tile
DK = D // P                 # 16 contraction chunks
NBT = T // PT               # 4 phase-1 token tiles per batch
NPT = NTOK // PT            # 16 total
DIAG = QT // KC             # 4
NQT = T // QT               # 4
TBB = T // P                # 16 token blocks per batch
NOD = D // QT               # 4

_CACHE = {}


def _build_module():
    import concourse.bass as bass  # noqa: F401
    import concourse.mybir as mybir
    from concourse import bacc
    import concourse.tile as tile

    F32 = mybir.dt.float32
    F16 = mybir.dt.float16
    BF16 = mybir.dt.bfloat16
    AF = mybir.ActivationFunctionType

    SCALE = 1.0 / float(np.sqrt(HD))

    nc = bacc.Bacc("TRN2", target_bir_lowering=False, debug=False)

    # All host-prepped layouts are partition-contiguous (one descriptor
    # per partition per DMA).
    x_s = nc.dram_tensor("x_s", [P, NPT, DK, PT], BF16, kind="ExternalInput").ap()
    wq_s = nc.dram_tensor("wq_s", [P, DK, DPC], BF16, kind="ExternalInput").ap()
    wk_s = nc.dram_tensor("wk_s", [P, DK, DPC], BF16, kind="ExternalInput").ap()
    wv_s = nc.dram_tensor("wv_s", [P, DK, DPC], BF16, kind="ExternalInput").ap()
    wo_s = nc.dram_tensor("wo_s", [P, HPC, D], BF16, kind="ExternalInput").ap()
    mk_s = nc.dram_tensor("mk_s", [P, DIAG, QT], BF16, kind="ExternalInput").ap()

    kT_out = nc.dram_tensor("kT_out", [DPC, NTOK], BF16, kind="ExternalOutput").ap()
    v_out = nc.dram_tensor("v_out", [P, NTOK // P, DPC], BF16,
                           kind="ExternalOutput").ap()
    out_p = nc.dram_tensor("out_p", [P, NTOK // P, D], BF16,
                           kind="ExternalOutput").ap()

    with tile.TileContext(nc) as tc:
        with (
            tc.tile_pool(name="w", bufs=1) as w_pool,
            tc.tile_pool(name="cst", bufs=1) as cst_pool,
            tc.tile_pool(name="xt", bufs=4) as xt_pool,
            tc.tile_pool(name="qk", bufs=2) as qk_pool,
            tc.tile_pool(name="vv", bufs=2) as v_pool,
            tc.tile_pool(name="ctx", bufs=2) as ctx_pool,
            tc.tile_pool(name="et", bufs=6) as et_pool,
            tc.tile_pool(name="Ep", bufs=2) as E_pool,
            tc.tile_pool(name="dn", bufs=2) as dn_pool,
            tc.tile_pool(name="st3", bufs=3) as st3_pool,
            tc.tile_pool(name="pa", bufs=3, space="PSUM") as pp_a,
            tc.tile_pool(name="ps", bufs=3, space="PSUM") as pp_s,
            tc.tile_pool(name="pc", bufs=2, space="PSUM") as pp_ctx,
        ):
            xt_tiles = {}

            def load_xt(b, tb):
                t_ = xt_pool.tile([P, DK, PT], BF16, tag="xt")
                nc.sync.dma_start(t_[:], x_s[:, b * NBT + tb, :, :])
                xt_tiles[(b, tb)] = t_

            # wq + first x tile first so the PE can start ASAP.
            wq_sb = w_pool.tile([P, DK, DPC], BF16, tag="wq")
            nc.sync.dma_start(wq_sb[:], wq_s)
            load_xt(0, 0)
            wk_sb = w_pool.tile([P, DK, DPC], BF16, tag="wk")
            nc.sync.dma_start(wk_sb[:], wk_s)
            wv_sb = w_pool.tile([P, DK, DPC], BF16, tag="wv")
            nc.sync.dma_start(wv_sb[:], wv_s)
            load_xt(0, 1)
            wo_sb = w_pool.tile([P, HPC, D], BF16, tag="wo")
            nc.sync.dma_start(wo_sb[:], wo_s)
            mask_sb = cst_pool.tile([P, DIAG, QT], BF16, tag="mask")
            nc.sync.dma_start(mask_sb[:], mk_s)
            load_xt(0, 2)
            load_xt(0, 3)
            ones_sb = cst_pool.tile([P, P], F16, tag="ones")
            nc.vector.memset(ones_sb[:], 1.0)

            qkv_tiles = {}
            ctx_tiles = {}
            pending_blocks = []

            def ph1_unit(b, u):
                # unit u: token-tile u//2; half 0 = q chains, half 1 = k
                # chains; each with 2 of the 4 v sub-chains.
                if u == 0:
                    qkv_tiles[b] = (
                        qk_pool.tile([P, HPC, T], BF16, tag="q", name="qT"),
                        qk_pool.tile([P, HPC, T], BF16, tag="k", name="kT"),
                        v_pool.tile([P, TBB, DPC], BF16, tag="v", name="vb"),
                    )
                qT, kT, vb = qkv_tiles[b]
                tb, half = divmod(u, 2)
                xt = xt_tiles[(b, tb)]
                w_sb, dst = (wq_sb, qT) if half == 0 else (wk_sb, kT)
                for hc in range(HPC):
                    ps = pp_a.tile([P, PT], F32, tag="pa")
                    for dc in range(DK):
                        nc.tensor.matmul(
                            ps[:], w_sb[:, dc, hc * P:(hc + 1) * P],
                            xt[:, dc, :],
                            start=(dc == 0), stop=(dc == DK - 1))
                    nc.scalar.copy(dst[:, hc, tb * PT:(tb + 1) * PT], ps[:])
                for sub in ((0, 1) if half == 0 else (2, 3)):
                    ps = pp_a.tile([P, PT], F32, tag="pa")
                    for dc in range(DK):
                        nc.tensor.matmul(
                            ps[:, :DPC], xt[:, dc, sub * P:(sub + 1) * P],
                            wv_sb[:, dc, :],
                            start=(dc == 0), stop=(dc == DK - 1))
                    nc.vector.tensor_copy(
                        vb[:, tb * (PT // P) + sub, :], ps[:, :DPC])
                if u == 7:
                    for h in range(HPC):
                        nc.sync.dma_start(
                            kT_out[h * P:(h + 1) * P, b * T:(b + 1) * T],
                            kT[:, h, :])
                    nc.sync.dma_start(
                        v_out[:, b * TBB:(b + 1) * TBB, :], vb[:])

            def ph3_block(b, tb2):
                ctxb = ctx_tiles[b]
                ost = st3_pool.tile([P, D], BF16, tag="ost")
                ts2 = slice(tb2 * P, (tb2 + 1) * P)
                for od in range(NOD):
                    ods = slice(od * QT, (od + 1) * QT)
                    ps = pp_a.tile([P, PT], F32, tag="pa")
                    nc.tensor.matmul(ps[:], ctxb[:, 0, ts2],
                                     wo_sb[:, 0, ods], start=True, stop=False)
                    nc.tensor.matmul(ps[:], ctxb[:, 1, ts2],
                                     wo_sb[:, 1, ods], start=False, stop=True)
                    if od % 2 == 0:
                        nc.vector.tensor_copy(ost[:, ods], ps[:])
                    else:
                        nc.scalar.copy(ost[:, ods], ps[:])
                nc.sync.dma_start(out_p[:, b * TBB + tb2, :], ost[:])

            def drain_block():
                if pending_blocks:
                    bb, tb2 = pending_blocks.pop(0)
                    ph3_block(bb, tb2)

            def ph2_group(b, g):
                qt, h = divmod(g, 2)
                if g == 0:
                    ctx_tiles[b] = ctx_pool.tile(
                        [P, HPC, T], BF16, tag="ctx", name="ctxb")
                ctxb = ctx_tiles[b]
                qT, kT, vb = qkv_tiles[b]
                nkc = (qt + 1) * DIAG
                qs = slice(qt * QT, (qt + 1) * QT)
                ctx_ps = pp_ctx.tile([P, QT], F32, tag="pc")
                E = E_pool.tile([P, QT], F16, tag="E")
                ets = {}

                def emit_scores(kc):
                    s_ps = pp_s.tile([P, QT], F32, tag="ps")
                    nc.tensor.matmul(
                        s_ps[:], kT[:, h, kc * KC:(kc + 1) * KC],
                        qT[:, h, qs], start=True, stop=True)
                    e_t = et_pool.tile([P, QT], BF16, tag="et")
                    nc.scalar.activation(e_t[:], s_ps[:], AF.Exp, scale=SCALE)
                    j = kc - qt * DIAG
                    if j >= 0:
                        nc.vector.tensor_mul(e_t[:], e_t[:], mask_sb[:, j, :])
                    if kc == 0:
                        nc.vector.tensor_copy(E[:], e_t[:])
                    else:
                        nc.vector.tensor_add(E[:], E[:], e_t[:])
                    ets[kc] = e_t

                # lookahead-2 so exp latency of chunk kc hides under the
                # scores matmuls of kc+1/kc+2.
                emit_scores(0)
                if nkc > 1:
                    emit_scores(1)
                for kc in range(nkc):
                    if kc + 2 < nkc:
                        emit_scores(kc + 2)
                    nc.tensor.matmul(
                        ctx_ps[:], vb[:, kc, h * HD:(h + 1) * HD],
                        ets.pop(kc)[:],
                        start=(kc == 0), stop=(kc == nkc - 1))
                    if (kc + 1) % 5 == 0:
                        drain_block()
                den_ps = pp_s.tile([P, QT], F32, tag="ps")
                nc.tensor.matmul(den_ps[:], ones_sb[:], E[:],
                                 start=True, stop=True)
                rc = dn_pool.tile([P, QT], F32, tag="rc")
                nc.vector.reciprocal(rc[:], den_ps[:])
                nc.vector.tensor_mul(ctxb[:, h, qs], ctx_ps[:], rc[:])
                drain_block()
                if h == 1:
                    for tb2 in range(qt * (QT // P), (qt + 1) * (QT // P)):
                        pending_blocks.append((b, tb2))

            # ---- schedule ----
            for u in range(8):
                ph1_unit(0, u)
                if u == 4:
                    load_xt(1, 0)
                if u == 6:
                    load_xt(1, 1)
            for b in range(B):
                for g in range(8):
                    if b + 1 < B:
                        ph1_unit(b + 1, g)
                        if g == 0:
                            load_xt(b + 1, 2)
                        if g == 1:
                            load_xt(b + 1, 3)
                        if g == 4 and b + 2 < B:
                            load_xt(b + 2, 0)
                        if g == 5 and b + 2 < B:
                            load_xt(b + 2, 1)
                    ph2_group(b, g)
            while pending_blocks:
                drain_block()

    nc.compile()
    return nc


def _get_module():
    if "nc" not in _CACHE:
        _CACHE["nc"] = _build_module()
    return _CACHE["nc"]


def _make_masks():
    m = np.zeros((DIAG, KC, QT), dtype=np.float32)
    for j in range(DIAG):
        for kk in range(KC):
            m[j, kk, j * KC + kk:] = 1.0
    return m


def _run(x, Wq, Wk, Wv, Wo, bo, trace=False):
    from concourse import bass_utils
    import ml_dtypes

    BF16 = ml_dtypes.bfloat16
    nc = _get_module()

    x2 = np.asarray(x, np.float32).reshape(NTOK, D).astype(BF16)
    # x_s[p, gt, dc, t] = x[gt*PT + t, dc*P + p]
    x_s = np.ascontiguousarray(
        x2.reshape(NPT, PT, DK, P).transpose(3, 0, 2, 1))
    masks = _make_masks().transpose(1, 0, 2)  # [P, DIAG, QT]
    masks = np.ascontiguousarray(masks.astype(BF16))

    Wq = np.asarray(Wq, np.float32)
    Wk = np.asarray(Wk, np.float32)
    Wv = np.asarray(Wv, np.float32)
    Wo = np.asarray(Wo, np.float32)

    def prep_w(Wslice):  # [DPC, D] -> [P, DK, DPC], w[p, dk, n] = W.T[dk*P+p, n]
        wT = Wslice.T.astype(BF16)  # [D, DPC]
        return np.ascontiguousarray(wT.reshape(DK, P, DPC).transpose(1, 0, 2))

    in_maps = []
    for c in range(N_CORES):
        sl = slice(c * DPC, (c + 1) * DPC)
        woT = Wo[:, sl].T.astype(BF16)  # [DPC, D]
        in_maps.append({
            "x_s": x_s,
            "wq_s": prep_w(Wq[sl, :]),
            "wk_s": prep_w(Wk[sl, :]),
            "wv_s": prep_w(Wv[sl, :]),
            "wo_s": np.ascontiguousarray(
                woT.reshape(HPC, P, D).transpose(1, 0, 2)),
            "mk_s": masks,
        })

    res = bass_utils.run_bass_kernel_spmd(
        nc, in_maps, core_ids=list(range(N_CORES)), trace=trace)

    out = np.zeros((NTOK, D), np.float32)
    k = np.empty((NTOK, D), np.float32)
    v = np.empty((NTOK, D), np.float32)
    for c, r in enumerate(res.results):
        sl = slice(c * DPC, (c + 1) * DPC)
        out += np.asarray(r["out_p"], np.float32).transpose(1, 0, 2).reshape(
            NTOK, D)
        k[:, sl] = np.asarray(r["kT_out"], np.float32).T
        v[:, sl] = np.asarray(r["v_out"], np.float32).transpose(
            1, 0, 2).reshape(NTOK, DPC)
    out += np.asarray(bo, np.float32)[None, :]
    outs = (out.reshape(B, T, D), k.reshape(B, T, D), v.reshape(B, T, D))
    return outs, res


def kernel(x, Wq, Wk, Wv, Wo, bo):
    outs, _ = _run(x, Wq, Wk, Wv, Wo, bo, trace=False)
    return outs


# revision 10
# speedup vs baseline: 1.3398x; 1.0243x over previous
"""Trainium2 Bass kernel for nn_MultiHeadAttention (B=4, T=2048, D=2048, H=16).

Sharding: tensor-parallel over heads; each of 8 NeuronCores owns 2 heads
(256 of 2048 q/k/v dims).  All matmul operands are bf16 (f32 PSUM
accumulation); inputs are cast host-side, outputs are written bf16 and
upcast/summed host-side.

Per core, three phases, kept fully SBUF-resident (no DRAM scratch):
  ph1(b): qT/kT projections in [head_dim, tok] layout and v in [tok, dim],
          streaming partition-contiguous x tiles from HBM.
  ph2(b): per (q-tile, head): scoresT[ktok, qtok] = kT_chunk.T @ qT, exp on
          ScalarE (no max-subtraction; logits are O(1) by construction),
          causal block-skip + diagonal masks on DVE, AV accumulation.
          Softmax denominator: DVE accumulates per-partition partial sums
          E[ktok%128, qtok] in fp16, then one ones-matmul per q-tile
          reduces over partitions (instead of a matmul per k-chunk).
  ph3(b): out_partial[tok, :] = sum_h (ctx_h/den_h).T @ WoT_h per
          128-token block, staged PSUM->SBUF (split DVE/ScalarE), written
          bf16.
The schedule interleaves three streams to keep the PE queue dense:
ph1(b+1) units alternate with ph2(b) groups, and ph3 blocks are drip-fed
into ph2's chunk loops to cover the exp (ScalarE) latency.
Host: Wo partials summed across cores in f32; k/v slices concatenated.
"""

import os
import sys

import numpy as np

for _p in ("/opt/trn_rl_repo",):
    if _p not in sys.path and os.path.isdir(_p):
        sys.path.insert(0, _p)

B, T, D, H = 4, 2048, 2048, 16
HD = 128
N_CORES = 8
HPC = H // N_CORES          # heads per core
DPC = HPC * HD              # q/k/v dims per core
NTOK = B * T

P = 128
QT = 512                    # q-tile width
KC = 128                    # k-chunk
PT = 512                    # phase-1 token tile
DK = D // P                 # 16 contraction chunks
NBT = T // PT               # 4 phase-1 token tiles per batch
NPT = NTOK // PT            # 16 total
DIAG = QT // KC             # 4
NQT = T // QT               # 4
TBB = T // P                # 16 token blocks per batch
NOD = D // QT               # 4

_CACHE = {}


def _build_module():
    import concourse.bass as bass  # noqa: F401
    import concourse.mybir as mybir
    from concourse import bacc
    import concourse.tile as tile

    F32 = mybir.dt.float32
    F16 = mybir.dt.float16
    BF16 = mybir.dt.bfloat16
    AF = mybir.ActivationFunctionType

    SCALE = 1.0 / float(np.sqrt(HD))

    nc = bacc.Bacc("TRN2", target_bir_lowering=False, debug=False)

    # All host-prepped layouts are partition-contiguous (one descriptor
    # per partition per DMA).
    x_s = nc.dram_tensor("x_s", [P, NPT, DK, PT], BF16, kind="ExternalInput").ap()
    wq_s = nc.dram_tensor("wq_s", [P, DK, DPC], BF16, kind="ExternalInput").ap()
    wk_s = nc.dram_tensor("wk_s", [P, DK, DPC], BF16, kind="ExternalInput").ap()
    wv_s = nc.dram_tensor("wv_s", [P, DK, DPC], BF16, kind="ExternalInput").ap()
    wo_s = nc.dram_tensor("wo_s", [P, HPC, D], BF16, kind="ExternalInput").ap()
    # tri[kk, c] = 1 iff c >= kk: causal mask for the 128x128 diagonal
    # sub-block of any diagonal chunk.
    mk_s = nc.dram_tensor("mk_s", [P, KC], BF16, kind="ExternalInput").ap()

    kT_out = nc.dram_tensor("kT_out", [DPC, NTOK], BF16, kind="ExternalOutput").ap()
    v_out = nc.dram_tensor("v_out", [P, NTOK // P, DPC], BF16,
                           kind="ExternalOutput").ap()
    out_p = nc.dram_tensor("out_p", [P, NTOK // P, D], BF16,
                           kind="ExternalOutput").ap()

    with tile.TileContext(nc) as tc:
        with (
            tc.tile_pool(name="w", bufs=1) as w_pool,
            tc.tile_pool(name="cst", bufs=1) as cst_pool,
            tc.tile_pool(name="xt", bufs=4) as xt_pool,
            tc.tile_pool(name="qk", bufs=2) as qk_pool,
            tc.tile_pool(name="vv", bufs=2) as v_pool,
            tc.tile_pool(name="ctx", bufs=2) as ctx_pool,
            tc.tile_pool(name="et", bufs=6) as et_pool,
            tc.tile_pool(name="Ep", bufs=2) as E_pool,
            tc.tile_pool(name="dn", bufs=2) as dn_pool,
            tc.tile_pool(name="st3", bufs=3) as st3_pool,
            tc.tile_pool(name="pa", bufs=3, space="PSUM") as pp_a,
            tc.tile_pool(name="ps", bufs=3, space="PSUM") as pp_s,
            tc.tile_pool(name="pc", bufs=2, space="PSUM") as pp_ctx,
        ):
            xt_tiles = {}

            def load_xt(b, tb):
                t_ = xt_pool.tile([P, DK, PT], BF16, tag="xt")
                nc.sync.dma_start(t_[:], x_s[:, b * NBT + tb, :, :])
                xt_tiles[(b, tb)] = t_

            # wq + first x tile first so the PE can start ASAP.
            wq_sb = w_pool.tile([P, DK, DPC], BF16, tag="wq")
            nc.sync.dma_start(wq_sb[:], wq_s)
            load_xt(0, 0)
            wk_sb = w_pool.tile([P, DK, DPC], BF16, tag="wk")
            nc.sync.dma_start(wk_sb[:], wk_s)
            wv_sb = w_pool.tile([P, DK, DPC], BF16, tag="wv")
            nc.sync.dma_start(wv_sb[:], wv_s)
            load_xt(0, 1)
            wo_sb = w_pool.tile([P, HPC, D], BF16, tag="wo")
            nc.sync.dma_start(wo_sb[:], wo_s)
            mask_sb = cst_pool.tile([P, KC], BF16, tag="mask")
            nc.sync.dma_start(mask_sb[:], mk_s)
            load_xt(0, 2)
            load_xt(0, 3)
            ones_sb = cst_pool.tile([P, P], F16, tag="ones")
            nc.vector.memset(ones_sb[:], 1.0)

            qkv_tiles = {}
            ctx_tiles = {}
            pending_blocks = []

            def ph1_unit(b, u):
                # unit u: token-tile u//2; half 0 = q chains, half 1 = k
                # chains; each with 2 of the 4 v sub-chains.
                if u == 0:
                    qkv_tiles[b] = (
                        qk_pool.tile([P, HPC, T], BF16, tag="q", name="qT"),
                        qk_pool.tile([P, HPC, T], BF16, tag="k", name="kT"),
                        v_pool.tile([P, TBB, DPC], BF16, tag="v", name="vb"),
                    )
                qT, kT, vb = qkv_tiles[b]
                tb, half = divmod(u, 2)
                xt = xt_tiles[(b, tb)]
                w_sb, dst = (wq_sb, qT) if half == 0 else (wk_sb, kT)
                for hc in range(HPC):
                    ps = pp_a.tile([P, PT], F32, tag="pa")
                    for dc in range(DK):
                        nc.tensor.matmul(
                            ps[:], w_sb[:, dc, hc * P:(hc + 1) * P],
                            xt[:, dc, :],
                            start=(dc == 0), stop=(dc == DK - 1))
                    nc.scalar.copy(dst[:, hc, tb * PT:(tb + 1) * PT], ps[:])
                for sub in ((0, 1) if half == 0 else (2, 3)):
                    ps = pp_a.tile([P, PT], F32, tag="pa")
                    for dc in range(DK):
                        nc.tensor.matmul(
                            ps[:, :DPC], xt[:, dc, sub * P:(sub + 1) * P],
                            wv_sb[:, dc, :],
                            start=(dc == 0), stop=(dc == DK - 1))
                    nc.vector.tensor_copy(
                        vb[:, tb * (PT // P) + sub, :], ps[:, :DPC])
                if u == 7:
                    for h in range(HPC):
                        nc.sync.dma_start(
                            kT_out[h * P:(h + 1) * P, b * T:(b + 1) * T],
                            kT[:, h, :])
                    nc.sync.dma_start(
                        v_out[:, b * TBB:(b + 1) * TBB, :], vb[:])

            def ph3_block(b, tb2):
                ctxb = ctx_tiles[b]
                ost = st3_pool.tile([P, D], BF16, tag="ost")
                ts2 = slice(tb2 * P, (tb2 + 1) * P)
                for od in range(NOD):
                    ods = slice(od * QT, (od + 1) * QT)
                    ps = pp_a.tile([P, PT], F32, tag="pa")
                    nc.tensor.matmul(ps[:], ctxb[:, 0, ts2],
                                     wo_sb[:, 0, ods], start=True, stop=False)
                    nc.tensor.matmul(ps[:], ctxb[:, 1, ts2],
                                     wo_sb[:, 1, ods], start=False, stop=True)
                    if od % 2 == 0:
                        nc.vector.tensor_copy(ost[:, ods], ps[:])
                    else:
                        nc.scalar.copy(ost[:, ods], ps[:])
                nc.sync.dma_start(out_p[:, b * TBB + tb2, :], ost[:])

            def drain_block():
                if pending_blocks:
                    bb, tb2 = pending_blocks.pop(0)
                    ph3_block(bb, tb2)

            def ph2_group(b, g):
                qt, h = divmod(g, 2)
                if g == 0:
                    ctx_tiles[b] = ctx_pool.tile(
                        [P, HPC, T], BF16, tag="ctx", name="ctxb")
                ctxb = ctx_tiles[b]
                qT, kT, vb = qkv_tiles[b]
                nkc = (qt + 1) * DIAG
                qs = slice(qt * QT, (qt + 1) * QT)
                ctx_ps = pp_ctx.tile([P, QT], F32, tag="pc")
                E = E_pool.tile([P, QT], F16, tag="E")
                ets = {}

                def emit_scores(kc):
                    # Diagonal chunk j: columns < j*KC are fully causal-
                    # masked; compute only [lo:], and apply the shared
                    # triangular mask to the single mixed 128-col sub-block.
                    j = kc - qt * DIAG
                    lo = j * KC if j > 0 else 0
                    s_ps = pp_s.tile([P, QT], F32, tag="ps")
                    nc.tensor.matmul(
                        s_ps[:, lo:], kT[:, h, kc * KC:(kc + 1) * KC],
                        qT[:, h, qt * QT + lo:(qt + 1) * QT],
                        start=True, stop=True)
                    e_t = et_pool.tile([P, QT], BF16, tag="et")
                    nc.scalar.activation(e_t[:, lo:], s_ps[:, lo:], AF.Exp,
                                         scale=SCALE)
                    if j >= 0:
                        nc.vector.tensor_mul(
                            e_t[:, lo:lo + KC], e_t[:, lo:lo + KC],
                            mask_sb[:])
                    if kc == 0:
                        nc.vector.tensor_copy(E[:], e_t[:])
                    else:
                        nc.vector.tensor_add(E[:, lo:], E[:, lo:],
                                             e_t[:, lo:])
                    ets[kc] = (e_t, lo)

                # lookahead-2 so exp latency of chunk kc hides under the
                # scores matmuls of kc+1/kc+2.
                emit_scores(0)
                if nkc > 1:
                    emit_scores(1)
                for kc in range(nkc):
                    if kc + 2 < nkc:
                        emit_scores(kc + 2)
                    e_t, lo = ets.pop(kc)
                    nc.tensor.matmul(
                        ctx_ps[:, lo:], vb[:, kc, h * HD:(h + 1) * HD],
                        e_t[:, lo:],
                        start=(kc == 0), stop=(kc == nkc - 1))
                    cadence = 4 if (len(pending_blocks) > 4 or b == B - 1) \
                        else 5
                    if (kc + 1) % cadence == 0:
                        drain_block()
                den_ps = pp_s.tile([P, QT], F32, tag="ps")
                nc.tensor.matmul(den_ps[:], ones_sb[:], E[:],
                                 start=True, stop=True)
                rc = dn_pool.tile([P, QT], F32, tag="rc")
                nc.vector.reciprocal(rc[:], den_ps[:])
                nc.vector.tensor_mul(ctxb[:, h, qs], ctx_ps[:], rc[:])
                drain_block()
                if len(pending_blocks) > 6:
                    drain_block()
                if h == 1:
                    for tb2 in range(qt * (QT // P), (qt + 1) * (QT // P)):
                        pending_blocks.append((b, tb2))

            # ---- schedule ----
            for u in range(8):
                ph1_unit(0, u)
                if u == 4:
                    load_xt(1, 0)
                if u == 6:
                    load_xt(1, 1)
            for b in range(B):
                for g in range(8):
                    if b + 1 < B:
                        ph1_unit(b + 1, g)
                        if g == 0:
                            load_xt(b + 1, 2)
                        if g == 1:
                            load_xt(b + 1, 3)
                        if g == 4 and b + 2 < B:
                            load_xt(b + 2, 0)
                        if g == 5 and b + 2 < B:
                            load_xt(b + 2, 1)
                    ph2_group(b, g)
            while pending_blocks:
                drain_block()

    nc.compile()
    return nc


def _get_module():
    if "nc" not in _CACHE:
        _CACHE["nc"] = _build_module()
    return _CACHE["nc"]


def _make_masks():
    # tri[kk, c] = 1 iff c >= kk
    return (np.arange(KC)[None, :] >= np.arange(KC)[:, None]).astype(
        np.float32)


def _run(x, Wq, Wk, Wv, Wo, bo, trace=False):
    from concourse import bass_utils
    import ml_dtypes

    BF16 = ml_dtypes.bfloat16
    nc = _get_module()

    x2 = np.asarray(x, np.float32).reshape(NTOK, D).astype(BF16)
    # x_s[p, gt, dc, t] = x[gt*PT + t, dc*P + p]
    x_s = np.ascontiguousarray(
        x2.reshape(NPT, PT, DK, P).transpose(3, 0, 2, 1))
    masks = np.ascontiguousarray(_make_masks().astype(BF16))  # [P, KC]

    Wq = np.asarray(Wq, np.float32)
    Wk = np.asarray(Wk, np.float32)
    Wv = np.asarray(Wv, np.float32)
    Wo = np.asarray(Wo, np.float32)

    def prep_w(Wslice):  # [DPC, D] -> [P, DK, DPC], w[p, dk, n] = W.T[dk*P+p, n]
        wT = Wslice.T.astype(BF16)  # [D, DPC]
        return np.ascontiguousarray(wT.reshape(DK, P, DPC).transpose(1, 0, 2))

    in_maps = []
    for c in range(N_CORES):
        sl = slice(c * DPC, (c + 1) * DPC)
        woT = Wo[:, sl].T.astype(BF16)  # [DPC, D]
        in_maps.append({
            "x_s": x_s,
            "wq_s": prep_w(Wq[sl, :]),
            "wk_s": prep_w(Wk[sl, :]),
            "wv_s": prep_w(Wv[sl, :]),
            "wo_s": np.ascontiguousarray(
                woT.reshape(HPC, P, D).transpose(1, 0, 2)),
            "mk_s": masks,
        })

    res = bass_utils.run_bass_kernel_spmd(
        nc, in_maps, core_ids=list(range(N_CORES)), trace=trace)

    out = np.zeros((NTOK, D), np.float32)
    k = np.empty((NTOK, D), np.float32)
    v = np.empty((NTOK, D), np.float32)
    for c, r in enumerate(res.results):
        sl = slice(c * DPC, (c + 1) * DPC)
        out += np.asarray(r["out_p"], np.float32).transpose(1, 0, 2).reshape(
            NTOK, D)
        k[:, sl] = np.asarray(r["kT_out"], np.float32).T
        v[:, sl] = np.asarray(r["v_out"], np.float32).transpose(
            1, 0, 2).reshape(NTOK, DPC)
    out += np.asarray(bo, np.float32)[None, :]
    outs = (out.reshape(B, T, D), k.reshape(B, T, D), v.reshape(B, T, D))
    return outs, res


def kernel(x, Wq, Wk, Wv, Wo, bo):
    outs, _ = _run(x, Wq, Wk, Wv, Wo, bo, trace=False)
    return outs


# revision 14
# speedup vs baseline: 1.4134x; 1.0549x over previous
"""Trainium2 Bass kernel for nn_MultiHeadAttention (B=4, T=2048, D=2048, H=16).

Sharding: tensor-parallel over heads; each of 8 NeuronCores owns 2 heads
(256 of 2048 q/k/v dims).  All matmul operands are bf16 (f32 PSUM
accumulation); inputs are cast host-side, outputs are written bf16 and
upcast/summed host-side.

Per core, three phases, kept fully SBUF-resident (no DRAM scratch):
  ph1(b): qT/kT projections in [head_dim, tok] layout and v in [tok, dim],
          streaming partition-contiguous x tiles from HBM.
  ph2(b): per (q-tile, head): scoresT[ktok, qtok] = kT_chunk.T @ qT, exp on
          ScalarE (no max-subtraction; logits are O(1) by construction),
          causal block-skip + diagonal masks on DVE, AV accumulation.
          Softmax denominator: DVE accumulates per-partition partial sums
          E[ktok%128, qtok] in fp16, then one ones-matmul per q-tile
          reduces over partitions (instead of a matmul per k-chunk).
  ph3(b): out_partial[tok, :] = sum_h (ctx_h/den_h).T @ WoT_h per
          128-token block, staged PSUM->SBUF (split DVE/ScalarE), written
          bf16.
The schedule interleaves three streams to keep the PE queue dense:
ph1(b+1) units alternate with ph2(b) groups, and ph3 blocks are drip-fed
into ph2's chunk loops to cover the exp (ScalarE) latency.
Host: Wo partials summed across cores in f32; k/v slices concatenated.
"""

import os
import sys

import numpy as np

for _p in ("/opt/trn_rl_repo",):
    if _p not in sys.path and os.path.isdir(_p):
        sys.path.insert(0, _p)

B, T, D, H = 4, 2048, 2048, 16
HD = 128
N_CORES = 8
HPC = H // N_CORES          # heads per core
DPC = HPC * HD              # q/k/v dims per core
NTOK = B * T

P = 128
QT = 512                    # q-tile width
KC = 128                    # k-chunk
PT = 512                    # phase-1 token tile
DK = D // P                 # 16 contraction chunks
NBT = T // PT               # 4 phase-1 token tiles per batch
NPT = NTOK // PT            # 16 total
DIAG = QT // KC             # 4
NQT = T // QT               # 4
TBB = T // P                # 16 token blocks per batch
NOD = D // QT               # 4

_CACHE = {}


def _build_module():
    import concourse.bass as bass  # noqa: F401
    import concourse.mybir as mybir
    from concourse import bacc
    import concourse.tile as tile

    F32 = mybir.dt.float32
    F16 = mybir.dt.float16
    BF16 = mybir.dt.bfloat16
    AF = mybir.ActivationFunctionType

    SCALE = 1.0 / float(np.sqrt(HD))

    nc = bacc.Bacc("TRN2", target_bir_lowering=False, debug=False)

    # All host-prepped layouts are partition-contiguous (one descriptor
    # per partition per DMA).
    x_s = nc.dram_tensor("x_s", [P, NPT, DK, PT], BF16, kind="ExternalInput").ap()
    wq_s = nc.dram_tensor("wq_s", [P, DK, DPC], BF16, kind="ExternalInput").ap()
    wk_s = nc.dram_tensor("wk_s", [P, DK, DPC], BF16, kind="ExternalInput").ap()
    wv_s = nc.dram_tensor("wv_s", [P, DK, DPC], BF16, kind="ExternalInput").ap()
    wo_s = nc.dram_tensor("wo_s", [P, HPC, D], BF16, kind="ExternalInput").ap()
    # tri[kk, c] = 1 iff c >= kk: causal mask for the 128x128 diagonal
    # sub-block of any diagonal chunk.
    mk_s = nc.dram_tensor("mk_s", [P, KC], BF16, kind="ExternalInput").ap()

    kT_out = nc.dram_tensor("kT_out", [DPC, NTOK], BF16, kind="ExternalOutput").ap()
    v_out = nc.dram_tensor("v_out", [P, NTOK // P, DPC], BF16,
                           kind="ExternalOutput").ap()
    out_p = nc.dram_tensor("out_p", [P, NTOK // P, D], BF16,
                           kind="ExternalOutput").ap()

    with tile.TileContext(nc) as tc:
        with (
            tc.tile_pool(name="w", bufs=1) as w_pool,
            tc.tile_pool(name="cst", bufs=1) as cst_pool,
            tc.tile_pool(name="xt", bufs=4) as xt_pool,
            tc.tile_pool(name="qk", bufs=2) as qk_pool,
            tc.tile_pool(name="vv", bufs=2) as v_pool,
            tc.tile_pool(name="ctx", bufs=2) as ctx_pool,
            tc.tile_pool(name="et", bufs=6) as et_pool,
            tc.tile_pool(name="Ep", bufs=2) as E_pool,
            tc.tile_pool(name="dn", bufs=2) as dn_pool,
            tc.tile_pool(name="st3", bufs=3) as st3_pool,
            tc.tile_pool(name="pa", bufs=3, space="PSUM") as pp_a,
            tc.tile_pool(name="ps", bufs=3, space="PSUM") as pp_s,
            tc.tile_pool(name="pc", bufs=2, space="PSUM") as pp_ctx,
        ):
            xt_tiles = {}

            def load_xt(b, tb, eng=None):
                t_ = xt_pool.tile([P, DK, PT], BF16, tag="xt")
                (eng or nc.sync).dma_start(t_[:], x_s[:, b * NBT + tb, :, :])
                xt_tiles[(b, tb)] = t_

            # wq on the sync hwdge queue, first x tile concurrently on the
            # scalar hwdge queue, so the PE can start ASAP.
            wq_sb = w_pool.tile([P, DK, DPC], BF16, tag="wq")
            nc.sync.dma_start(wq_sb[:], wq_s)
            load_xt(0, 0, eng=nc.scalar)
            wk_sb = w_pool.tile([P, DK, DPC], BF16, tag="wk")
            nc.sync.dma_start(wk_sb[:], wk_s)
            wv_sb = w_pool.tile([P, DK, DPC], BF16, tag="wv")
            nc.sync.dma_start(wv_sb[:], wv_s)
            load_xt(0, 1, eng=nc.scalar)
            wo_sb = w_pool.tile([P, HPC, D], BF16, tag="wo")
            nc.sync.dma_start(wo_sb[:], wo_s)
            mask_sb = cst_pool.tile([P, KC], BF16, tag="mask")
            nc.sync.dma_start(mask_sb[:], mk_s)
            load_xt(0, 2)
            load_xt(0, 3)
            ones_sb = cst_pool.tile([P, P], F16, tag="ones")
            nc.vector.memset(ones_sb[:], 1.0)

            qkv_tiles = {}
            ctx_tiles = {}
            pending_blocks = []

            def ph1_unit(b, u):
                # unit u: token-tile u//2; half 0 = q chains, half 1 = k
                # chains; each with 2 of the 4 v sub-chains.
                if u == 0:
                    qkv_tiles[b] = (
                        qk_pool.tile([P, HPC, T], BF16, tag="q", name="qT"),
                        qk_pool.tile([P, HPC, T], BF16, tag="k", name="kT"),
                        v_pool.tile([P, TBB, DPC], BF16, tag="v", name="vb"),
                    )
                qT, kT, vb = qkv_tiles[b]
                tb, half = divmod(u, 2)
                xt = xt_tiles[(b, tb)]
                w_sb, dst = (wq_sb, qT) if half == 0 else (wk_sb, kT)
                for hc in range(HPC):
                    ps = pp_a.tile([P, PT], F32, tag="pa")
                    for dc in range(DK):
                        nc.tensor.matmul(
                            ps[:], w_sb[:, dc, hc * P:(hc + 1) * P],
                            xt[:, dc, :],
                            start=(dc == 0), stop=(dc == DK - 1))
                    nc.scalar.copy(dst[:, hc, tb * PT:(tb + 1) * PT], ps[:])
                for sub in ((0, 1) if half == 0 else (2, 3)):
                    ps = pp_a.tile([P, PT], F32, tag="pa")
                    for dc in range(DK):
                        nc.tensor.matmul(
                            ps[:, :DPC], xt[:, dc, sub * P:(sub + 1) * P],
                            wv_sb[:, dc, :],
                            start=(dc == 0), stop=(dc == DK - 1))
                    nc.vector.tensor_copy(
                        vb[:, tb * (PT // P) + sub, :], ps[:, :DPC])
                if u == 7:
                    for h in range(HPC):
                        nc.sync.dma_start(
                            kT_out[h * P:(h + 1) * P, b * T:(b + 1) * T],
                            kT[:, h, :])
                    nc.sync.dma_start(
                        v_out[:, b * TBB:(b + 1) * TBB, :], vb[:])

            def ph3_block(b, tb2):
                ctxb = ctx_tiles[b]
                ost = st3_pool.tile([P, D], BF16, tag="ost")
                ts2 = slice(tb2 * P, (tb2 + 1) * P)
                for od in range(NOD):
                    ods = slice(od * QT, (od + 1) * QT)
                    ps = pp_a.tile([P, PT], F32, tag="pa")
                    nc.tensor.matmul(ps[:], ctxb[:, 0, ts2],
                                     wo_sb[:, 0, ods], start=True, stop=False)
                    nc.tensor.matmul(ps[:], ctxb[:, 1, ts2],
                                     wo_sb[:, 1, ods], start=False, stop=True)
                    if od % 2 == 0:
                        nc.vector.tensor_copy(ost[:, ods], ps[:])
                    else:
                        nc.scalar.copy(ost[:, ods], ps[:])
                nc.sync.dma_start(out_p[:, b * TBB + tb2, :], ost[:])

            def drain_block():
                if pending_blocks:
                    bb, tb2 = pending_blocks.pop(0)
                    ph3_block(bb, tb2)

            def ph2_group(b, qt, h, first):
                if first:
                    ctx_tiles[b] = ctx_pool.tile(
                        [P, HPC, T], BF16, tag="ctx", name="ctxb")
                ctxb = ctx_tiles[b]
                qT, kT, vb = qkv_tiles[b]
                nkc = (qt + 1) * DIAG
                qs = slice(qt * QT, (qt + 1) * QT)
                ctx_ps = pp_ctx.tile([P, QT], F32, tag="pc")
                E = E_pool.tile([P, QT], F16, tag="E")
                ets = {}

                def emit_scores(kc):
                    # Diagonal chunk j: columns < j*KC are fully causal-
                    # masked; compute only [lo:], and apply the shared
                    # triangular mask to the single mixed 128-col sub-block.
                    j = kc - qt * DIAG
                    lo = j * KC if j > 0 else 0
                    s_ps = pp_s.tile([P, QT], F32, tag="ps")
                    nc.tensor.matmul(
                        s_ps[:, lo:], kT[:, h, kc * KC:(kc + 1) * KC],
                        qT[:, h, qt * QT + lo:(qt + 1) * QT],
                        start=True, stop=True)
                    e_t = et_pool.tile([P, QT], BF16, tag="et")
                    nc.scalar.activation(e_t[:, lo:], s_ps[:, lo:], AF.Exp,
                                         scale=SCALE)
                    if j >= 0:
                        nc.vector.tensor_mul(
                            e_t[:, lo:lo + KC], e_t[:, lo:lo + KC],
                            mask_sb[:])
                    if kc == 0:
                        nc.vector.tensor_copy(E[:], e_t[:])
                    else:
                        nc.vector.tensor_add(E[:, lo:], E[:, lo:],
                                             e_t[:, lo:])
                    ets[kc] = (e_t, lo)

                # lookahead-2 so exp latency of chunk kc hides under the
                # scores matmuls of kc+1/kc+2.
                emit_scores(0)
                if nkc > 1:
                    emit_scores(1)
                for kc in range(nkc):
                    if kc + 2 < nkc:
                        emit_scores(kc + 2)
                    e_t, lo = ets.pop(kc)
                    nc.tensor.matmul(
                        ctx_ps[:, lo:], vb[:, kc, h * HD:(h + 1) * HD],
                        e_t[:, lo:],
                        start=(kc == 0), stop=(kc == nkc - 1))
                    cadence = 4 if (len(pending_blocks) > 4 or b == B - 1) \
                        else 5
                    if (kc + 1) % cadence == 0:
                        drain_block()
                den_ps = pp_s.tile([P, QT], F32, tag="ps")
                nc.tensor.matmul(den_ps[:], ones_sb[:], E[:],
                                 start=True, stop=True)
                rc = dn_pool.tile([P, QT], F32, tag="rc")
                # den > 0 always (diagonal term e^s(q,q) is present), so the
                # fast approx (~18 correct bits) is safe and plenty.
                nc.vector.reciprocal_approx_fast(out=rc[:], in_=den_ps[:])
                nc.vector.tensor_mul(ctxb[:, h, qs], ctx_ps[:], rc[:])
                drain_block()
                if len(pending_blocks) > 6:
                    drain_block()
                if h == 1:
                    for tb2 in range(qt * (QT // P), (qt + 1) * (QT // P)):
                        pending_blocks.append((b, tb2))

            # ---- schedule ----
            for u in range(8):
                ph1_unit(0, u)
                if u == 4:
                    load_xt(1, 0)
                if u == 6:
                    load_xt(1, 1)
            for b in range(B):
                # Last batch: largest q-tiles first, so the final output
                # blocks are gated on the shortest group.
                qts = list(range(NQT)) if b + 1 < B else \
                    list(reversed(range(NQT)))
                groups = [(qt, h) for qt in qts for h in range(HPC)]
                for g, (qt, h) in enumerate(groups):
                    if b + 1 < B:
                        ph1_unit(b + 1, g)
                        if g == 0:
                            load_xt(b + 1, 2)
                        if g == 1:
                            load_xt(b + 1, 3)
                        if g == 4 and b + 2 < B:
                            load_xt(b + 2, 0)
                        if g == 5 and b + 2 < B:
                            load_xt(b + 2, 1)
                    ph2_group(b, qt, h, first=(g == 0))
            while pending_blocks:
                drain_block()

    nc.compile()
    return nc


def _get_module():
    if "nc" not in _CACHE:
        _CACHE["nc"] = _build_module()
    return _CACHE["nc"]


def _make_masks():
    # tri[kk, c] = 1 iff c >= kk
    return (np.arange(KC)[None, :] >= np.arange(KC)[:, None]).astype(
        np.float32)


def _run(x, Wq, Wk, Wv, Wo, bo, trace=False):
    from concourse import bass_utils
    import ml_dtypes

    BF16 = ml_dtypes.bfloat16
    nc = _get_module()

    x2 = np.asarray(x, np.float32).reshape(NTOK, D).astype(BF16)
    # x_s[p, gt, dc, t] = x[gt*PT + t, dc*P + p]
    x_s = np.ascontiguousarray(
        x2.reshape(NPT, PT, DK, P).transpose(3, 0, 2, 1))
    masks = np.ascontiguousarray(_make_masks().astype(BF16))  # [P, KC]

    Wq = np.asarray(Wq, np.float32)
    Wk = np.asarray(Wk, np.float32)
    Wv = np.asarray(Wv, np.float32)
    Wo = np.asarray(Wo, np.float32)

    def prep_w(Wslice):  # [DPC, D] -> [P, DK, DPC], w[p, dk, n] = W.T[dk*P+p, n]
        wT = Wslice.T.astype(BF16)  # [D, DPC]
        return np.ascontiguousarray(wT.reshape(DK, P, DPC).transpose(1, 0, 2))

    in_maps = []
    for c in range(N_CORES):
        sl = slice(c * DPC, (c + 1) * DPC)
        woT = Wo[:, sl].T.astype(BF16)  # [DPC, D]
        in_maps.append({
            "x_s": x_s,
            "wq_s": prep_w(Wq[sl, :]),
            "wk_s": prep_w(Wk[sl, :]),
            "wv_s": prep_w(Wv[sl, :]),
            "wo_s": np.ascontiguousarray(
                woT.reshape(HPC, P, D).transpose(1, 0, 2)),
            "mk_s": masks,
        })

    res = bass_utils.run_bass_kernel_spmd(
        nc, in_maps, core_ids=list(range(N_CORES)), trace=trace)

    out = np.zeros((NTOK, D), np.float32)
    k = np.empty((NTOK, D), np.float32)
    v = np.empty((NTOK, D), np.float32)
    for c, r in enumerate(res.results):
        sl = slice(c * DPC, (c + 1) * DPC)
        out += np.asarray(r["out_p"], np.float32).transpose(1, 0, 2).reshape(
            NTOK, D)
        k[:, sl] = np.asarray(r["kT_out"], np.float32).T
        v[:, sl] = np.asarray(r["v_out"], np.float32).transpose(
            1, 0, 2).reshape(NTOK, DPC)
    out += np.asarray(bo, np.float32)[None, :]
    outs = (out.reshape(B, T, D), k.reshape(B, T, D), v.reshape(B, T, D))
    return outs, res


def kernel(x, Wq, Wk, Wv, Wo, bo):
    outs, _ = _run(x, Wq, Wk, Wv, Wo, bo, trace=False)
    return outs


# revision 17
# speedup vs baseline: 1.4146x; 1.0008x over previous
"""Trainium2 Bass kernel for nn_MultiHeadAttention (B=4, T=2048, D=2048, H=16).

Sharding: tensor-parallel over heads; each of 8 NeuronCores owns 2 heads
(256 of 2048 q/k/v dims).  All matmul operands are bf16 (f32 PSUM
accumulation); inputs are cast host-side, outputs are written bf16 and
upcast/summed host-side.

Per core, three phases, kept fully SBUF-resident (no DRAM scratch):
  ph1(b): qT/kT projections in [head_dim, tok] layout and v in [tok, dim],
          streaming partition-contiguous x tiles from HBM.
  ph2(b): per (q-tile, head): scoresT[ktok, qtok] = kT_chunk.T @ qT, exp on
          ScalarE (no max-subtraction; logits are O(1) by construction),
          causal block-skip + diagonal masks on DVE, AV accumulation.
          Softmax denominator: DVE accumulates per-partition partial sums
          E[ktok%128, qtok] in fp16, then one ones-matmul per q-tile
          reduces over partitions (instead of a matmul per k-chunk).
  ph3(b): out_partial[tok, :] = sum_h (ctx_h/den_h).T @ WoT_h per
          128-token block, staged PSUM->SBUF (split DVE/ScalarE), written
          bf16.
The schedule interleaves three streams to keep the PE queue dense:
ph1(b+1) units alternate with ph2(b) groups, and ph3 blocks are drip-fed
into ph2's chunk loops to cover the exp (ScalarE) latency.
Host: Wo partials summed across cores in f32; k/v slices concatenated.
"""

import os
import sys

import numpy as np

for _p in ("/opt/trn_rl_repo",):
    if _p not in sys.path and os.path.isdir(_p):
        sys.path.insert(0, _p)

B, T, D, H = 4, 2048, 2048, 16
HD = 128
N_CORES = 8
HPC = H // N_CORES          # heads per core
DPC = HPC * HD              # q/k/v dims per core
NTOK = B * T

P = 128
QT = 512                    # q-tile width
KC = 128                    # k-chunk
PT = 512                    # phase-1 token tile
DK = D // P                 # 16 contraction chunks
NBT = T // PT               # 4 phase-1 token tiles per batch
NPT = NTOK // PT            # 16 total
DIAG = QT // KC             # 4
NQT = T // QT               # 4
TBB = T // P                # 16 token blocks per batch
NOD = D // QT               # 4

_CACHE = {}


def _build_module():
    import concourse.bass as bass  # noqa: F401
    import concourse.mybir as mybir
    from concourse import bacc
    import concourse.tile as tile

    F32 = mybir.dt.float32
    F16 = mybir.dt.float16
    BF16 = mybir.dt.bfloat16
    AF = mybir.ActivationFunctionType

    SCALE = 1.0 / float(np.sqrt(HD))

    nc = bacc.Bacc("TRN2", target_bir_lowering=False, debug=False)

    # All host-prepped layouts are partition-contiguous (one descriptor
    # per partition per DMA).
    x_s = nc.dram_tensor("x_s", [P, NPT, DK, PT], BF16, kind="ExternalInput").ap()
    wq_s = nc.dram_tensor("wq_s", [P, DK, DPC], BF16, kind="ExternalInput").ap()
    wk_s = nc.dram_tensor("wk_s", [P, DK, DPC], BF16, kind="ExternalInput").ap()
    wv_s = nc.dram_tensor("wv_s", [P, DK, DPC], BF16, kind="ExternalInput").ap()
    wo_s = nc.dram_tensor("wo_s", [P, HPC, D], BF16, kind="ExternalInput").ap()
    # tri[kk, c] = 1 iff c >= kk: causal mask for the 128x128 diagonal
    # sub-block of any diagonal chunk.
    mk_s = nc.dram_tensor("mk_s", [P, KC], BF16, kind="ExternalInput").ap()

    kT_out = nc.dram_tensor("kT_out", [DPC, NTOK], BF16, kind="ExternalOutput").ap()
    v_out = nc.dram_tensor("v_out", [P, NTOK // P, DPC], BF16,
                           kind="ExternalOutput").ap()
    out_p = nc.dram_tensor("out_p", [P, NTOK // P, D], BF16,
                           kind="ExternalOutput").ap()

    with tile.TileContext(nc) as tc:
        with (
            tc.tile_pool(name="w", bufs=1) as w_pool,
            tc.tile_pool(name="cst", bufs=1) as cst_pool,
            tc.tile_pool(name="xt", bufs=4) as xt_pool,
            tc.tile_pool(name="qk", bufs=2) as qk_pool,
            tc.tile_pool(name="vv", bufs=2) as v_pool,
            tc.tile_pool(name="ctx", bufs=2) as ctx_pool,
            tc.tile_pool(name="et", bufs=8) as et_pool,
            tc.tile_pool(name="Ep", bufs=2) as E_pool,
            tc.tile_pool(name="dn", bufs=2) as dn_pool,
            tc.tile_pool(name="st3", bufs=3) as st3_pool,
            tc.tile_pool(name="pa", bufs=3, space="PSUM") as pp_a,
            tc.tile_pool(name="ps", bufs=3, space="PSUM") as pp_s,
            tc.tile_pool(name="pc", bufs=2, space="PSUM") as pp_ctx,
        ):
            xt_tiles = {}

            def load_xt(b, tb, eng=None):
                # 4 region sub-DMAs per tile: with subtile deps, projection
                # chains start as soon as their dc-range lands.
                t_ = xt_pool.tile([P, DK, PT], BF16, tag="xt")
                gt = b * NBT + tb
                for dg in range(0, DK, 4):
                    (eng or nc.sync).dma_start(
                        t_[:, dg:dg + 4, :], x_s[:, gt, dg:dg + 4, :])
                xt_tiles[(b, tb)] = t_

            def load_w(dst, src, eng):
                for dg in range(0, DK, 8):
                    eng.dma_start(dst[:, dg:dg + 8, :], src[:, dg:dg + 8, :])

            # Interleave wq/wv (sync hwdge queue) with the first x tile
            # (scalar hwdge queue) so unit 0 (q + v chains) starts ASAP;
            # wk is only needed by unit 1.
            wq_sb = w_pool.tile([P, DK, DPC], BF16, tag="wq")
            load_w(wq_sb, wq_s, nc.sync)
            load_xt(0, 0, eng=nc.scalar)
            wv_sb = w_pool.tile([P, DK, DPC], BF16, tag="wv")
            load_w(wv_sb, wv_s, nc.sync)
            wk_sb = w_pool.tile([P, DK, DPC], BF16, tag="wk")
            load_w(wk_sb, wk_s, nc.sync)
            load_xt(0, 1, eng=nc.scalar)
            wo_sb = w_pool.tile([P, HPC, D], BF16, tag="wo")
            nc.sync.dma_start(wo_sb[:], wo_s)
            mask_sb = cst_pool.tile([P, KC], BF16, tag="mask")
            nc.sync.dma_start(mask_sb[:], mk_s)
            load_xt(0, 2)
            load_xt(0, 3)
            ones_sb = cst_pool.tile([P, P], F16, tag="ones")
            nc.vector.memset(ones_sb[:], 1.0)

            qkv_tiles = {}
            ctx_tiles = {}
            pending_blocks = []

            def ph1_unit(b, u):
                # unit u: token-tile u//2; half 0 = q chains, half 1 = k
                # chains; each with 2 of the 4 v sub-chains.
                if u == 0:
                    qkv_tiles[b] = (
                        qk_pool.tile([P, HPC, T], BF16, tag="q", name="qT"),
                        qk_pool.tile([P, HPC, T], BF16, tag="k", name="kT"),
                        v_pool.tile([P, TBB, DPC], BF16, tag="v", name="vb"),
                    )
                qT, kT, vb = qkv_tiles[b]
                tb, half = divmod(u, 2)
                xt = xt_tiles[(b, tb)]
                w_sb, dst = (wq_sb, qT) if half == 0 else (wk_sb, kT)
                for hc in range(HPC):
                    ps = pp_a.tile([P, PT], F32, tag="pa")
                    for dc in range(DK):
                        nc.tensor.matmul(
                            ps[:], w_sb[:, dc, hc * P:(hc + 1) * P],
                            xt[:, dc, :],
                            start=(dc == 0), stop=(dc == DK - 1))
                    nc.scalar.copy(dst[:, hc, tb * PT:(tb + 1) * PT], ps[:])
                for sub in ((0, 1) if half == 0 else (2, 3)):
                    ps = pp_a.tile([P, PT], F32, tag="pa")
                    for dc in range(DK):
                        nc.tensor.matmul(
                            ps[:, :DPC], xt[:, dc, sub * P:(sub + 1) * P],
                            wv_sb[:, dc, :],
                            start=(dc == 0), stop=(dc == DK - 1))
                    nc.vector.tensor_copy(
                        vb[:, tb * (PT // P) + sub, :], ps[:, :DPC])
                if u == 7:
                    for h in range(HPC):
                        nc.sync.dma_start(
                            kT_out[h * P:(h + 1) * P, b * T:(b + 1) * T],
                            kT[:, h, :])
                    nc.sync.dma_start(
                        v_out[:, b * TBB:(b + 1) * TBB, :], vb[:])

            def ph3_block(b, tb2):
                ctxb = ctx_tiles[b]
                ost = st3_pool.tile([P, D], BF16, tag="ost")
                ts2 = slice(tb2 * P, (tb2 + 1) * P)
                for od in range(NOD):
                    ods = slice(od * QT, (od + 1) * QT)
                    ps = pp_a.tile([P, PT], F32, tag="pa")
                    nc.tensor.matmul(ps[:], ctxb[:, 0, ts2],
                                     wo_sb[:, 0, ods], start=True, stop=False)
                    nc.tensor.matmul(ps[:], ctxb[:, 1, ts2],
                                     wo_sb[:, 1, ods], start=False, stop=True)
                    if od % 2 == 0:
                        nc.vector.tensor_copy(ost[:, ods], ps[:])
                    else:
                        nc.scalar.copy(ost[:, ods], ps[:])
                nc.sync.dma_start(out_p[:, b * TBB + tb2, :], ost[:])

            def drain_block():
                if pending_blocks:
                    bb, tb2 = pending_blocks.pop(0)
                    ph3_block(bb, tb2)

            def ph2_group(b, qt, h, first):
                if first:
                    ctx_tiles[b] = ctx_pool.tile(
                        [P, HPC, T], BF16, tag="ctx", name="ctxb")
                ctxb = ctx_tiles[b]
                qT, kT, vb = qkv_tiles[b]
                nkc = (qt + 1) * DIAG
                qs = slice(qt * QT, (qt + 1) * QT)
                ctx_ps = pp_ctx.tile([P, QT], F32, tag="pc")
                E = E_pool.tile([P, QT], F16, tag="E")
                ets = {}

                def emit_scores(kc):
                    # Diagonal chunk j: columns < j*KC are fully causal-
                    # masked; compute only [lo:], and apply the shared
                    # triangular mask to the single mixed 128-col sub-block.
                    j = kc - qt * DIAG
                    lo = j * KC if j > 0 else 0
                    s_ps = pp_s.tile([P, QT], F32, tag="ps")
                    nc.tensor.matmul(
                        s_ps[:, lo:], kT[:, h, kc * KC:(kc + 1) * KC],
                        qT[:, h, qt * QT + lo:(qt + 1) * QT],
                        start=True, stop=True)
                    e_t = et_pool.tile([P, QT], BF16, tag="et")
                    nc.scalar.activation(e_t[:, lo:], s_ps[:, lo:], AF.Exp,
                                         scale=SCALE)
                    if j >= 0:
                        nc.vector.tensor_mul(
                            e_t[:, lo:lo + KC], e_t[:, lo:lo + KC],
                            mask_sb[:])
                    if kc == 0:
                        nc.vector.tensor_copy(E[:], e_t[:])
                    else:
                        nc.vector.tensor_add(E[:, lo:], E[:, lo:],
                                             e_t[:, lo:])
                    ets[kc] = (e_t, lo)

                # lookahead-3 so exp latency of chunk kc hides under the
                # scores matmuls of kc+1..kc+3.
                for ka in range(min(3, nkc)):
                    emit_scores(ka)
                for kc in range(nkc):
                    if kc + 3 < nkc:
                        emit_scores(kc + 3)
                    e_t, lo = ets.pop(kc)
                    nc.tensor.matmul(
                        ctx_ps[:, lo:], vb[:, kc, h * HD:(h + 1) * HD],
                        e_t[:, lo:],
                        start=(kc == 0), stop=(kc == nkc - 1))
                    cadence = 4 if (len(pending_blocks) > 4 or b == B - 1) \
                        else 5
                    if (kc + 1) % cadence == 0:
                        drain_block()
                den_ps = pp_s.tile([P, QT], F32, tag="ps")
                nc.tensor.matmul(den_ps[:], ones_sb[:], E[:],
                                 start=True, stop=True)
                rc = dn_pool.tile([P, QT], F32, tag="rc")
                # den > 0 always (diagonal term e^s(q,q) is present), so the
                # fast approx (~18 correct bits) is safe and plenty.
                nc.vector.reciprocal_approx_fast(out=rc[:], in_=den_ps[:])
                nc.vector.tensor_mul(ctxb[:, h, qs], ctx_ps[:], rc[:])
                drain_block()
                if len(pending_blocks) > 6:
                    drain_block()
                if h == 1:
                    for tb2 in range(qt * (QT // P), (qt + 1) * (QT // P)):
                        pending_blocks.append((b, tb2))

            # ---- schedule ----
            for u in range(8):
                ph1_unit(0, u)
                if u == 4:
                    load_xt(1, 0)
                if u == 6:
                    load_xt(1, 1)
            for b in range(B):
                # Last batch: largest q-tiles first, so the final output
                # blocks are gated on the shortest group.
                qts = list(range(NQT)) if b + 1 < B else \
                    list(reversed(range(NQT)))
                groups = [(qt, h) for qt in qts for h in range(HPC)]
                for g, (qt, h) in enumerate(groups):
                    if b + 1 < B:
                        ph1_unit(b + 1, g)
                        if g == 0:
                            load_xt(b + 1, 2)
                        if g == 1:
                            load_xt(b + 1, 3)
                        if g == 4 and b + 2 < B:
                            load_xt(b + 2, 0)
                        if g == 5 and b + 2 < B:
                            load_xt(b + 2, 1)
                    ph2_group(b, qt, h, first=(g == 0))
            while pending_blocks:
                drain_block()

    nc.compile()
    return nc


def _get_module():
    if "nc" not in _CACHE:
        _CACHE["nc"] = _build_module()
    return _CACHE["nc"]


def _make_masks():
    # tri[kk, c] = 1 iff c >= kk
    return (np.arange(KC)[None, :] >= np.arange(KC)[:, None]).astype(
        np.float32)


def _run(x, Wq, Wk, Wv, Wo, bo, trace=False):
    from concourse import bass_utils
    import ml_dtypes

    BF16 = ml_dtypes.bfloat16
    nc = _get_module()

    x2 = np.asarray(x, np.float32).reshape(NTOK, D).astype(BF16)
    # x_s[p, gt, dc, t] = x[gt*PT + t, dc*P + p]
    x_s = np.ascontiguousarray(
        x2.reshape(NPT, PT, DK, P).transpose(3, 0, 2, 1))
    masks = np.ascontiguousarray(_make_masks().astype(BF16))  # [P, KC]

    Wq = np.asarray(Wq, np.float32)
    Wk = np.asarray(Wk, np.float32)
    Wv = np.asarray(Wv, np.float32)
    Wo = np.asarray(Wo, np.float32)

    def prep_w(Wslice):  # [DPC, D] -> [P, DK, DPC], w[p, dk, n] = W.T[dk*P+p, n]
        wT = Wslice.T.astype(BF16)  # [D, DPC]
        return np.ascontiguousarray(wT.reshape(DK, P, DPC).transpose(1, 0, 2))

    in_maps = []
    for c in range(N_CORES):
        sl = slice(c * DPC, (c + 1) * DPC)
        woT = Wo[:, sl].T.astype(BF16)  # [DPC, D]
        in_maps.append({
            "x_s": x_s,
            "wq_s": prep_w(Wq[sl, :]),
            "wk_s": prep_w(Wk[sl, :]),
            "wv_s": prep_w(Wv[sl, :]),
            "wo_s": np.ascontiguousarray(
                woT.reshape(HPC, P, D).transpose(1, 0, 2)),
            "mk_s": masks,
        })

    res = bass_utils.run_bass_kernel_spmd(
        nc, in_maps, core_ids=list(range(N_CORES)), trace=trace)

    out = np.zeros((NTOK, D), np.float32)
    k = np.empty((NTOK, D), np.float32)
    v = np.empty((NTOK, D), np.float32)
    for c, r in enumerate(res.results):
        sl = slice(c * DPC, (c + 1) * DPC)
        out += np.asarray(r["out_p"], np.float32).transpose(1, 0, 2).reshape(
            NTOK, D)
        k[:, sl] = np.asarray(r["kT_out"], np.float32).T
        v[:, sl] = np.asarray(r["v_out"], np.float32).transpose(
            1, 0, 2).reshape(NTOK, DPC)
    out += np.asarray(bo, np.float32)[None, :]
    outs = (out.reshape(B, T, D), k.reshape(B, T, D), v.reshape(B, T, D))
    return outs, res


def kernel(x, Wq, Wk, Wv, Wo, bo):
    outs, _ = _run(x, Wq, Wk, Wv, Wo, bo, trace=False)
    return outs


# revision 18
# speedup vs baseline: 1.4300x; 1.0109x over previous
"""Trainium2 Bass kernel for nn_MultiHeadAttention (B=4, T=2048, D=2048, H=16).

Sharding: tensor-parallel over heads; each of 8 NeuronCores owns 2 heads
(256 of 2048 q/k/v dims).  All matmul operands are bf16 (f32 PSUM
accumulation); inputs are cast host-side, outputs are written bf16 and
upcast/summed host-side.

Per core, three phases, kept fully SBUF-resident (no DRAM scratch):
  ph1(b): qT/kT projections in [head_dim, tok] layout and v in [tok, dim],
          streaming partition-contiguous x tiles from HBM.
  ph2(b): per (q-tile, head): scoresT[ktok, qtok] = kT_chunk.T @ qT, exp on
          ScalarE (no max-subtraction; logits are O(1) by construction),
          causal block-skip + diagonal masks on DVE, AV accumulation.
          Softmax denominator: DVE accumulates per-partition partial sums
          E[ktok%128, qtok] in fp16, then one ones-matmul per q-tile
          reduces over partitions (instead of a matmul per k-chunk).
  ph3(b): out_partial[tok, :] = sum_h (ctx_h/den_h).T @ WoT_h per
          128-token block, staged PSUM->SBUF (split DVE/ScalarE), written
          bf16.
The schedule interleaves three streams to keep the PE queue dense:
ph1(b+1) units alternate with ph2(b) groups, and ph3 blocks are drip-fed
into ph2's chunk loops to cover the exp (ScalarE) latency.
Host: Wo partials summed across cores in f32; k/v slices concatenated.
"""

import os
import sys

import numpy as np

for _p in ("/opt/trn_rl_repo",):
    if _p not in sys.path and os.path.isdir(_p):
        sys.path.insert(0, _p)

B, T, D, H = 4, 2048, 2048, 16
HD = 128
N_CORES = 8
HPC = H // N_CORES          # heads per core
DPC = HPC * HD              # q/k/v dims per core
NTOK = B * T

P = 128
QT = 512                    # q-tile width
KC = 128                    # k-chunk
PT = 512                    # phase-1 token tile
DK = D // P                 # 16 contraction chunks
NBT = T // PT               # 4 phase-1 token tiles per batch
NPT = NTOK // PT            # 16 total
DIAG = QT // KC             # 4
NQT = T // QT               # 4
TBB = T // P                # 16 token blocks per batch
NOD = D // QT               # 4

_CACHE = {}


def _build_module():
    import concourse.bass as bass  # noqa: F401
    import concourse.mybir as mybir
    from concourse import bacc
    import concourse.tile as tile

    F32 = mybir.dt.float32
    F16 = mybir.dt.float16
    BF16 = mybir.dt.bfloat16
    AF = mybir.ActivationFunctionType

    SCALE = 1.0 / float(np.sqrt(HD))

    nc = bacc.Bacc("TRN2", target_bir_lowering=False, debug=False)

    # All host-prepped layouts are partition-contiguous (one descriptor
    # per partition per DMA).
    x_s = nc.dram_tensor("x_s", [P, NPT, DK, PT], BF16, kind="ExternalInput").ap()
    wq_s = nc.dram_tensor("wq_s", [P, DK, DPC], BF16, kind="ExternalInput").ap()
    wk_s = nc.dram_tensor("wk_s", [P, DK, DPC], BF16, kind="ExternalInput").ap()
    wv_s = nc.dram_tensor("wv_s", [P, DK, DPC], BF16, kind="ExternalInput").ap()
    wo_s = nc.dram_tensor("wo_s", [P, HPC, D], BF16, kind="ExternalInput").ap()
    # tri[kk, c] = 1 iff c >= kk: causal mask for the 128x128 diagonal
    # sub-block of any diagonal chunk.
    mk_s = nc.dram_tensor("mk_s", [P, KC], BF16, kind="ExternalInput").ap()

    kT_out = nc.dram_tensor("kT_out", [DPC, NTOK], BF16, kind="ExternalOutput").ap()
    v_out = nc.dram_tensor("v_out", [P, NTOK // P, DPC], BF16,
                           kind="ExternalOutput").ap()
    out_p = nc.dram_tensor("out_p", [P, NTOK // P, D], BF16,
                           kind="ExternalOutput").ap()

    with tile.TileContext(nc) as tc:
        with (
            tc.tile_pool(name="w", bufs=1) as w_pool,
            tc.tile_pool(name="cst", bufs=1) as cst_pool,
            tc.tile_pool(name="xt", bufs=4) as xt_pool,
            tc.tile_pool(name="qk", bufs=2) as qk_pool,
            tc.tile_pool(name="vv", bufs=2) as v_pool,
            tc.tile_pool(name="ctx", bufs=2) as ctx_pool,
            tc.tile_pool(name="et", bufs=8) as et_pool,
            tc.tile_pool(name="Ep", bufs=2) as E_pool,
            tc.tile_pool(name="dn", bufs=2) as dn_pool,
            tc.tile_pool(name="st3", bufs=3) as st3_pool,
            tc.tile_pool(name="pa", bufs=3, space="PSUM") as pp_a,
            tc.tile_pool(name="ps", bufs=3, space="PSUM") as pp_s,
            tc.tile_pool(name="pc", bufs=2, space="PSUM") as pp_ctx,
        ):
            xt_tiles = {}

            def load_xt(b, tb, eng=None):
                # 4 region sub-DMAs per tile: with subtile deps, projection
                # chains start as soon as their dc-range lands.
                t_ = xt_pool.tile([P, DK, PT], BF16, tag="xt")
                gt = b * NBT + tb
                for dg in range(0, DK, 4):
                    (eng or nc.sync).dma_start(
                        t_[:, dg:dg + 4, :], x_s[:, gt, dg:dg + 4, :])
                xt_tiles[(b, tb)] = t_

            def load_w(dst, src, eng):
                for dg in range(0, DK, 8):
                    eng.dma_start(dst[:, dg:dg + 8, :], src[:, dg:dg + 8, :])

            # Interleave wq/wv (sync hwdge queue) with the first x tile
            # (scalar hwdge queue) so unit 0 (q + v chains) starts ASAP;
            # wk is only needed by unit 1.
            wq_sb = w_pool.tile([P, DK, DPC], BF16, tag="wq")
            load_w(wq_sb, wq_s, nc.sync)
            load_xt(0, 0, eng=nc.scalar)
            wv_sb = w_pool.tile([P, DK, DPC], BF16, tag="wv")
            load_w(wv_sb, wv_s, nc.sync)
            wk_sb = w_pool.tile([P, DK, DPC], BF16, tag="wk")
            load_w(wk_sb, wk_s, nc.sync)
            load_xt(0, 1, eng=nc.scalar)
            wo_sb = w_pool.tile([P, HPC, D], BF16, tag="wo")
            nc.sync.dma_start(wo_sb[:], wo_s)
            mask_sb = cst_pool.tile([P, KC], BF16, tag="mask")
            nc.sync.dma_start(mask_sb[:], mk_s)
            load_xt(0, 2)
            load_xt(0, 3)
            ones_sb = cst_pool.tile([P, P], F16, tag="ones")
            nc.vector.memset(ones_sb[:], 1.0)

            qkv_tiles = {}
            ctx_tiles = {}
            pending_blocks = []

            def ph1_unit(b, u):
                # unit u: token-tile u//2; half 0 = q chains, half 1 = k
                # chains; each with 2 of the 4 v sub-chains.
                if u == 0:
                    qkv_tiles[b] = (
                        qk_pool.tile([P, HPC, T], BF16, tag="q", name="qT"),
                        qk_pool.tile([P, HPC, T], BF16, tag="k", name="kT"),
                        v_pool.tile([P, TBB, DPC], BF16, tag="v", name="vb"),
                    )
                qT, kT, vb = qkv_tiles[b]
                tb, half = divmod(u, 2)
                xt = xt_tiles[(b, tb)]
                w_sb, dst = (wq_sb, qT) if half == 0 else (wk_sb, kT)
                for hc in range(HPC):
                    ps = pp_a.tile([P, PT], F32, tag="pa")
                    for dc in range(DK):
                        nc.tensor.matmul(
                            ps[:], w_sb[:, dc, hc * P:(hc + 1) * P],
                            xt[:, dc, :],
                            start=(dc == 0), stop=(dc == DK - 1))
                    nc.scalar.copy(dst[:, hc, tb * PT:(tb + 1) * PT], ps[:])
                for sub in ((0, 1) if half == 0 else (2, 3)):
                    ps = pp_a.tile([P, PT], F32, tag="pa")
                    for dc in range(DK):
                        nc.tensor.matmul(
                            ps[:, :DPC], xt[:, dc, sub * P:(sub + 1) * P],
                            wv_sb[:, dc, :],
                            start=(dc == 0), stop=(dc == DK - 1))
                    nc.vector.tensor_copy(
                        vb[:, tb * (PT // P) + sub, :], ps[:, :DPC])
                if u == 7:
                    for h in range(HPC):
                        nc.sync.dma_start(
                            kT_out[h * P:(h + 1) * P, b * T:(b + 1) * T],
                            kT[:, h, :])
                    nc.sync.dma_start(
                        v_out[:, b * TBB:(b + 1) * TBB, :], vb[:])

            def ph3_block(b, tb2):
                ctxb = ctx_tiles[b]
                ost = st3_pool.tile([P, D], BF16, tag="ost")
                ts2 = slice(tb2 * P, (tb2 + 1) * P)
                for od in range(NOD):
                    ods = slice(od * QT, (od + 1) * QT)
                    ps = pp_a.tile([P, PT], F32, tag="pa")
                    nc.tensor.matmul(ps[:], ctxb[:, 0, ts2],
                                     wo_sb[:, 0, ods], start=True, stop=False)
                    nc.tensor.matmul(ps[:], ctxb[:, 1, ts2],
                                     wo_sb[:, 1, ods], start=False, stop=True)
                    if od % 2 == 0:
                        nc.vector.tensor_copy(ost[:, ods], ps[:])
                    else:
                        nc.scalar.copy(ost[:, ods], ps[:])
                nc.sync.dma_start(out_p[:, b * TBB + tb2, :], ost[:])

            def drain_block():
                if pending_blocks:
                    bb, tb2 = pending_blocks.pop(0)
                    ph3_block(bb, tb2)

            def ph2_group(b, qt, h, first):
                if first:
                    ctx_tiles[b] = ctx_pool.tile(
                        [P, HPC, T], BF16, tag="ctx", name="ctxb")
                ctxb = ctx_tiles[b]
                qT, kT, vb = qkv_tiles[b]
                nkc = (qt + 1) * DIAG
                qs = slice(qt * QT, (qt + 1) * QT)
                ctx_ps = pp_ctx.tile([P, QT], F32, tag="pc")
                E = E_pool.tile([P, QT], F16, tag="E")
                ets = {}

                def emit_scores(kc):
                    # Diagonal chunk j: columns < j*KC are fully causal-
                    # masked; compute only [lo:], and apply the shared
                    # triangular mask to the single mixed 128-col sub-block.
                    j = kc - qt * DIAG
                    lo = j * KC if j > 0 else 0
                    s_ps = pp_s.tile([P, QT], F32, tag="ps")
                    nc.tensor.matmul(
                        s_ps[:, lo:], kT[:, h, kc * KC:(kc + 1) * KC],
                        qT[:, h, qt * QT + lo:(qt + 1) * QT],
                        start=True, stop=True)
                    e_t = et_pool.tile([P, QT], BF16, tag="et")
                    nc.scalar.activation(e_t[:, lo:], s_ps[:, lo:], AF.Exp,
                                         scale=SCALE)
                    if j >= 0:
                        nc.vector.tensor_mul(
                            e_t[:, lo:lo + KC], e_t[:, lo:lo + KC],
                            mask_sb[:])
                    if kc == 0:
                        nc.vector.tensor_copy(E[:], e_t[:])
                    else:
                        nc.vector.tensor_add(E[:, lo:], E[:, lo:],
                                             e_t[:, lo:])
                    ets[kc] = (e_t, lo)

                # lookahead-3 so exp latency of chunk kc hides under the
                # scores matmuls of kc+1..kc+3.
                for ka in range(min(3, nkc)):
                    emit_scores(ka)
                for kc in range(nkc):
                    if kc + 3 < nkc:
                        emit_scores(kc + 3)
                    # Drip a ph3 block in BEFORE the AV matmul so its
                    # matmuls fill the exp-latency window (critical in
                    # short groups where the scores matmuls are tiny).
                    if kc % 3 == 0:
                        drain_block()
                    e_t, lo = ets.pop(kc)
                    nc.tensor.matmul(
                        ctx_ps[:, lo:], vb[:, kc, h * HD:(h + 1) * HD],
                        e_t[:, lo:],
                        start=(kc == 0), stop=(kc == nkc - 1))
                den_ps = pp_s.tile([P, QT], F32, tag="ps")
                nc.tensor.matmul(den_ps[:], ones_sb[:], E[:],
                                 start=True, stop=True)
                rc = dn_pool.tile([P, QT], F32, tag="rc")
                # den > 0 always (diagonal term e^s(q,q) is present), so the
                # fast approx (~18 correct bits) is safe and plenty.
                nc.vector.reciprocal_approx_fast(out=rc[:], in_=den_ps[:])
                nc.vector.tensor_mul(ctxb[:, h, qs], ctx_ps[:], rc[:])
                drain_block()
                if len(pending_blocks) > 6:
                    drain_block()
                if h == 1:
                    for tb2 in range(qt * (QT // P), (qt + 1) * (QT // P)):
                        pending_blocks.append((b, tb2))

            # ---- schedule ----
            for u in range(8):
                ph1_unit(0, u)
                if u == 4:
                    load_xt(1, 0)
                if u == 6:
                    load_xt(1, 1)
            for b in range(B):
                # Last batch: largest q-tiles first, so the final output
                # blocks are gated on the shortest group.
                qts = list(range(NQT)) if b + 1 < B else \
                    list(reversed(range(NQT)))
                groups = [(qt, h) for qt in qts for h in range(HPC)]
                for g, (qt, h) in enumerate(groups):
                    if b + 1 < B:
                        ph1_unit(b + 1, g)
                        if g == 0:
                            load_xt(b + 1, 2)
                        if g == 1:
                            load_xt(b + 1, 3)
                        if g == 4 and b + 2 < B:
                            load_xt(b + 2, 0)
                        if g == 5 and b + 2 < B:
                            load_xt(b + 2, 1)
                    ph2_group(b, qt, h, first=(g == 0))
            while pending_blocks:
                drain_block()

    nc.compile()
    return nc


def _get_module():
    if "nc" not in _CACHE:
        _CACHE["nc"] = _build_module()
    return _CACHE["nc"]


def _make_masks():
    # tri[kk, c] = 1 iff c >= kk
    return (np.arange(KC)[None, :] >= np.arange(KC)[:, None]).astype(
        np.float32)


def _run(x, Wq, Wk, Wv, Wo, bo, trace=False):
    from concourse import bass_utils
    import ml_dtypes

    BF16 = ml_dtypes.bfloat16
    nc = _get_module()

    x2 = np.asarray(x, np.float32).reshape(NTOK, D).astype(BF16)
    # x_s[p, gt, dc, t] = x[gt*PT + t, dc*P + p]
    x_s = np.ascontiguousarray(
        x2.reshape(NPT, PT, DK, P).transpose(3, 0, 2, 1))
    masks = np.ascontiguousarray(_make_masks().astype(BF16))  # [P, KC]

    Wq = np.asarray(Wq, np.float32)
    Wk = np.asarray(Wk, np.float32)
    Wv = np.asarray(Wv, np.float32)
    Wo = np.asarray(Wo, np.float32)

    def prep_w(Wslice):  # [DPC, D] -> [P, DK, DPC], w[p, dk, n] = W.T[dk*P+p, n]
        wT = Wslice.T.astype(BF16)  # [D, DPC]
        return np.ascontiguousarray(wT.reshape(DK, P, DPC).transpose(1, 0, 2))

    in_maps = []
    for c in range(N_CORES):
        sl = slice(c * DPC, (c + 1) * DPC)
        woT = Wo[:, sl].T.astype(BF16)  # [DPC, D]
        in_maps.append({
            "x_s": x_s,
            "wq_s": prep_w(Wq[sl, :]),
            "wk_s": prep_w(Wk[sl, :]),
            "wv_s": prep_w(Wv[sl, :]),
            "wo_s": np.ascontiguousarray(
                woT.reshape(HPC, P, D).transpose(1, 0, 2)),
            "mk_s": masks,
        })

    res = bass_utils.run_bass_kernel_spmd(
        nc, in_maps, core_ids=list(range(N_CORES)), trace=trace)

    out = np.zeros((NTOK, D), np.float32)
    k = np.empty((NTOK, D), np.float32)
    v = np.empty((NTOK, D), np.float32)
    for c, r in enumerate(res.results):
        sl = slice(c * DPC, (c + 1) * DPC)
        out += np.asarray(r["out_p"], np.float32).transpose(1, 0, 2).reshape(
            NTOK, D)
        k[:, sl] = np.asarray(r["kT_out"], np.float32).T
        v[:, sl] = np.asarray(r["v_out"], np.float32).transpose(
            1, 0, 2).reshape(NTOK, DPC)
    out += np.asarray(bo, np.float32)[None, :]
    outs = (out.reshape(B, T, D), k.reshape(B, T, D), v.reshape(B, T, D))
    return outs, res


def kernel(x, Wq, Wk, Wv, Wo, bo):
    outs, _ = _run(x, Wq, Wk, Wv, Wo, bo, trace=False)
    return outs
